# revision 9
# baseline (speedup 1.0000x reference)
"""Trainium2 Bass kernel for ArcticMLP MoE grouped-GEMM (nn_ArcticMLPMoE).

Reference computation (per token group g of expert e, tokens sorted by expert):
    gate = x @ w1[e];  up = x @ w3[e];  out = (silu(gate) * up) @ w2[e]

Strategy
--------
Expert-parallel across the 8 NeuronCores: tokens arrive pre-sorted by
expert, so each core owns E/8 experts and their token slices -- zero
collectives.  The problem is weight-DMA bound (each weight byte is used
for only 128 tokens), so on the host we:
  * split tokens into 128-token buckets per expert (general ragged
    group_sizes supported via zero-padding; the standard case of 128
    tokens/expert is a pure reshape),
  * downcast weights/activations to bf16 (halves the HBM traffic;
    matmuls accumulate in fp32 PSUM, rel. error ~5e-3 << 2e-2),
  * pre-tile every tensor so each device DMA is fully contiguous.

Per bucket (128 tokens) the device streams w1/w3/w2 in F-chunks of 512:
    gate/up [128t x 512f] = sum_h xT[h,t].T @ w{1,3}[h,f]   (8 k-tiles)
    inter   = silu(gate) * up                  (ACT + DVE, fp32->bf16)
    interT  [f,t] via PE transpose (identity matmul)
    out    += interT.T @ w2[f,h]               (accumulated in PSUM)
"""

import os
import sys

import numpy as np

sys.path.insert(0, "/opt/trn_rl_repo")

E = 32
H = 1024
F = 2048
T = 4096
N_CORES = 8
TOK = 128          # tokens per bucket (= per expert in the standard case)
FC = 512           # F-chunk width (moving-operand free dim for gate/up)
N_FC = F // FC     # 4 chunks
HT = H // 128      # 8 k-tiles over hidden dim
FT = FC // 128     # 4 f-tiles per chunk

_COMPILED = {}     # buckets_per_core -> (nc, param_names)


def _build(nbpc: int):
    """Build + compile the per-core Bass graph for `nbpc` buckets/core."""
    from contextlib import ExitStack

    import concourse.bass as bass
    import concourse.mybir as mybir
    import concourse.tile as tile
    from concourse import bacc
    from concourse.masks import make_identity

    BF16 = mybir.dt.bfloat16
    F32 = mybir.dt.float32
    AF = mybir.ActivationFunctionType
    TPC = nbpc * TOK   # tokens per core

    nc = bacc.Bacc(
        "TRN2", target_bir_lowering=False, debug=False, num_devices=N_CORES
    )

    # Packed per-(bucket, F-chunk) weight block; per partition the free
    # axis is [w1c: HT*FC | w3c: HT*FC | w2c: FT*H] = 12288 bf16 = 24KB.
    WPF = 2 * HT * FC + FT * H
    xT_d = nc.dram_tensor("xt", [128, HT, TPC], BF16, kind="ExternalInput")
    wp_d = nc.dram_tensor("wp", [nbpc, N_FC, 128, WPF], BF16, kind="ExternalInput")
    out_d = nc.dram_tensor("out", [TPC, H], BF16, kind="ExternalOutput")
    W3_OFF = HT * FC
    W2_OFF = 2 * HT * FC

    with tile.TileContext(nc) as tc, ExitStack() as ctx:
        consts = ctx.enter_context(tc.tile_pool(name="consts", bufs=1))
        xpool = ctx.enter_context(tc.tile_pool(name="xpool", bufs=1))
        wpool = ctx.enter_context(tc.tile_pool(name="wpool", bufs=5))
        epool = ctx.enter_context(tc.tile_pool(name="epool", bufs=2))
        pg = ctx.enter_context(tc.tile_pool(name="pg", bufs=2, space="PSUM"))
        pt = ctx.enter_context(tc.tile_pool(name="pt", bufs=2, space="PSUM"))
        po = ctx.enter_context(tc.tile_pool(name="po", bufs=1, space="PSUM"))

        ident = consts.tile([128, 128], BF16)
        make_identity(nc, ident[:])

        xT = xpool.tile([128, HT, TPC], BF16)
        nc.sync.dma_start(out=xT[:], in_=xT_d[:])

        for b in range(nbpc):
            out_ps = po.tile([128, H], F32, tag="out_ps")
            for fc in range(N_FC):
                wc = wpool.tile([128, WPF], BF16, tag="wc")
                nc.sync.dma_start(out=wc[:], in_=wp_d[b, fc])

                gate = pg.tile([128, FC], F32, tag="gate")
                up = pg.tile([128, FC], F32, tag="up")
                for a in range(HT):
                    lhs = xT[:, a, b * TOK:(b + 1) * TOK]
                    nc.tensor.matmul(
                        gate[:], lhs, wc[:, a * FC:(a + 1) * FC],
                        start=(a == 0), stop=(a == HT - 1),
                    )
                    nc.tensor.matmul(
                        up[:], lhs, wc[:, W3_OFF + a * FC:W3_OFF + (a + 1) * FC],
                        start=(a == 0), stop=(a == HT - 1),
                    )

                silu = epool.tile([128, FC], F32, tag="silu")
                nc.scalar.activation(silu[:], gate[:], AF.Silu)
                inter = epool.tile([128, FC], BF16, tag="inter")
                nc.vector.tensor_mul(inter[:], silu[:], up[:])

                interT = epool.tile([128, FT, TOK], BF16, tag="interT")
                for ft in range(FT):
                    tps = pt.tile([128, TOK], BF16, tag="tps")
                    nc.tensor.transpose(
                        tps[:], inter[:, ft * 128:(ft + 1) * 128], ident[:]
                    )
                    nc.vector.tensor_copy(interT[:, ft, :], tps[:])

                for ft in range(FT):
                    first = fc == 0 and ft == 0
                    last = fc == N_FC - 1 and ft == FT - 1
                    for n in range(2):
                        off = W2_OFF + ft * H + n * 512
                        nc.tensor.matmul(
                            out_ps[:, n * 512:(n + 1) * 512],
                            interT[:, ft, :],
                            wc[:, off:off + 512],
                            start=first, stop=last,
                        )

            outs = epool.tile([128, H], BF16, tag="outs")
            nc.vector.tensor_copy(outs[:], out_ps[:])
            nc.sync.dma_start(out=out_d[b * TOK:(b + 1) * TOK, :], in_=outs[:])

    nc.compile()
    return nc


def _get_compiled(nbpc: int):
    if nbpc not in _COMPILED:
        _COMPILED[nbpc] = _build(nbpc)
    return _COMPILED[nbpc]


def _plan_buckets(group_sizes):
    """Split ragged expert groups into <=128-token buckets.

    Returns list of (expert_id, token_start, ntok)."""
    buckets = []
    start = 0
    for e, g in enumerate(np.asarray(group_sizes).astype(np.int64)):
        off = 0
        while off < g:
            n = min(TOK, g - off)
            buckets.append((e, start + off, int(n)))
            off += n
        start += int(g)
    return buckets


def _prepare_in_maps(hidden_states, w1, w3, w2, buckets, nbpc):
    import ml_dtypes

    bf16 = ml_dtypes.bfloat16
    nb = nbpc * N_CORES

    w1b = np.asarray(w1, dtype=bf16)
    w3b = np.asarray(w3, dtype=bf16)
    w2b = np.asarray(w2, dtype=bf16)
    hsb = np.asarray(hidden_states, dtype=bf16)

    # Token buckets: [nb, TOK, H], zero-padded.
    uniform = (
        len(buckets) == nb
        and all(n == TOK for (_, _, n) in buckets)
        and all(s == i * TOK for i, (_, s, _) in enumerate(buckets))
    )
    if uniform:
        xb = hsb.reshape(nb, TOK, H)
        eids = np.array([e for (e, _, _) in buckets])
    else:
        xb = np.zeros((nb, TOK, H), dtype=bf16)
        eids = np.zeros(nb, dtype=np.int64)
        for i, (e, s, n) in enumerate(buckets):
            xb[i, :n] = hsb[s:s + n]
            eids[i] = e

    # Per-bucket weights (gather; identity when one bucket per expert).
    w1g = w1b[eids]  # [nb, H, F]
    w3g = w3b[eids]
    w2g = w2b[eids]  # [nb, F, H]

    # Device layouts (everything contiguous per DMA):
    #  xT [128p(h%128), HT, TPC] per core
    #  wp [b, fc, 128p, w1c(HT,FC) | w3c(HT,FC) | w2c(FT,H)] packed per chunk
    #    w1/w3 partition = h%128, blocks indexed [h//128, f_in_chunk]
    #    w2    partition = f%128, blocks indexed [f//128 within chunk, h]
    w1t = w1g.reshape(nb, HT, 128, N_FC, FC).transpose(0, 3, 2, 1, 4)
    w3t = w3g.reshape(nb, HT, 128, N_FC, FC).transpose(0, 3, 2, 1, 4)
    w2t = w2g.reshape(nb, N_FC, FT, 128, H).transpose(0, 1, 3, 2, 4)
    wp = np.concatenate(
        [
            w1t.reshape(nb, N_FC, 128, HT * FC),
            w3t.reshape(nb, N_FC, 128, HT * FC),
            w2t.reshape(nb, N_FC, 128, FT * H),
        ],
        axis=3,
    )

    in_maps = []
    for c in range(N_CORES):
        sl = slice(c * nbpc, (c + 1) * nbpc)
        xc = xb[sl]  # [nbpc, TOK, H]
        # xT: [H, nbpc*TOK] -> [HT, 128, TPC] -> [128, HT, TPC]
        xt = np.ascontiguousarray(
            xc.reshape(nbpc * TOK, H).T.reshape(HT, 128, nbpc * TOK)
            .transpose(1, 0, 2)
        )
        in_maps.append({
            "xt": xt,
            "wp": np.ascontiguousarray(wp[sl]),
        })
    return in_maps


def _run(hidden_states, w1, w3, w2, group_sizes, trace=False, **run_kwargs):
    from concourse.bass_utils import run_bass_kernel_spmd

    buckets = _plan_buckets(group_sizes)
    nbpc = -(-len(buckets) // N_CORES)  # ceil
    nb = nbpc * N_CORES
    while len(buckets) < nb:
        buckets.append((0, 0, 0))  # padding buckets (zero tokens)

    nc = _get_compiled(nbpc)
    in_maps = _prepare_in_maps(hidden_states, w1, w3, w2, buckets, nbpc)
    res = run_bass_kernel_spmd(
        nc, in_maps, core_ids=list(range(N_CORES)), trace=trace, **run_kwargs
    )

    out_buckets = np.concatenate(
        [r["out"].astype(np.float32).reshape(nbpc, TOK, H) for r in res.results],
        axis=0,
    )  # [nb, TOK, H] float32

    T_total = int(np.asarray(group_sizes).sum())
    out = np.zeros((hidden_states.shape[0], H), dtype=np.float32)
    for i, (e, s, n) in enumerate(buckets):
        if n:
            out[s:s + n] = out_buckets[i, :n]
    del T_total
    return out, res


def kernel(hidden_states, w1, w3, w2, group_sizes):
    out, _ = _run(hidden_states, w1, w3, w2, group_sizes)
    return out


# revision 13
# speedup vs baseline: 1.0093x; 1.0093x over previous
"""Trainium2 Bass kernel for ArcticMLP MoE grouped-GEMM (nn_ArcticMLPMoE).

Reference computation (per token group g of expert e, tokens sorted by expert):
    gate = x @ w1[e];  up = x @ w3[e];  out = (silu(gate) * up) @ w2[e]

Strategy
--------
Expert-parallel across the 8 NeuronCores: tokens arrive pre-sorted by
expert, so each core owns E/8 experts and their token slices -- zero
collectives.  The problem is weight-DMA bound (each weight byte is used
for only 128 tokens), so on the host we:
  * split tokens into 128-token buckets per expert (general ragged
    group_sizes supported via zero-padding; the standard case of 128
    tokens/expert is a pure reshape),
  * downcast weights/activations to bf16 (halves the HBM traffic;
    matmuls accumulate in fp32 PSUM, rel. error ~5e-3 << 2e-2),
  * pre-tile every tensor so each device DMA is fully contiguous.

Per bucket (128 tokens) the device streams w1/w3/w2 in F-chunks of 512:
    gate/up [128t x 512f] = sum_h xT[h,t].T @ w{1,3}[h,f]   (8 k-tiles)
    inter   = silu(gate) * up                  (ACT + DVE, fp32->bf16)
    interT  [f,t] via PE transpose (identity matmul)
    out    += interT.T @ w2[f,h]               (accumulated in PSUM)
"""

import os
import sys

import numpy as np

sys.path.insert(0, "/opt/trn_rl_repo")

E = 32
H = 1024
F = 2048
T = 4096
N_CORES = 8
TOK = 128          # tokens per bucket (= per expert in the standard case)
FC = 512           # F-chunk width (moving-operand free dim for gate/up)
N_FC = F // FC     # 4 chunks
HT = H // 128      # 8 k-tiles over hidden dim
FT = FC // 128     # 4 f-tiles per chunk

_COMPILED = {}     # buckets_per_core -> (nc, param_names)


def _build(nbpc: int):
    """Build + compile the per-core Bass graph for `nbpc` buckets/core."""
    from contextlib import ExitStack

    import concourse.bass as bass
    import concourse.mybir as mybir
    import concourse.tile as tile
    from concourse import bacc
    from concourse.masks import make_identity

    BF16 = mybir.dt.bfloat16
    F32 = mybir.dt.float32
    AF = mybir.ActivationFunctionType
    TPC = nbpc * TOK   # tokens per core

    nc = bacc.Bacc(
        "TRN2", target_bir_lowering=False, debug=False, num_devices=N_CORES
    )

    xT_d = nc.dram_tensor("xt", [128, HT, TPC], BF16, kind="ExternalInput")
    w1_d = nc.dram_tensor("w1", [nbpc, N_FC, 128, HT, FC], BF16, kind="ExternalInput")
    w3_d = nc.dram_tensor("w3", [nbpc, N_FC, 128, HT, FC], BF16, kind="ExternalInput")
    w2_d = nc.dram_tensor("w2", [nbpc, N_FC, 128, FT, H], BF16, kind="ExternalInput")
    out_d = nc.dram_tensor("out", [TPC, H], BF16, kind="ExternalOutput")

    with tile.TileContext(nc) as tc, ExitStack() as ctx:
        consts = ctx.enter_context(tc.tile_pool(name="consts", bufs=1))
        xpool = ctx.enter_context(tc.tile_pool(name="xpool", bufs=1))
        wpool = ctx.enter_context(tc.tile_pool(name="wpool", bufs=5))
        epool = ctx.enter_context(tc.tile_pool(name="epool", bufs=2))
        pg = ctx.enter_context(tc.tile_pool(name="pg", bufs=2, space="PSUM"))
        pt = ctx.enter_context(tc.tile_pool(name="pt", bufs=2, space="PSUM"))
        po = ctx.enter_context(tc.tile_pool(name="po", bufs=1, space="PSUM"))

        ident = consts.tile([128, 128], BF16)
        make_identity(nc, ident[:])

        xT = xpool.tile([128, HT, TPC], BF16)
        nc.sync.dma_start(out=xT[:], in_=xT_d[:])

        for b in range(nbpc):
            out_ps = po.tile([128, H], F32, tag="out_ps")
            for fc in range(N_FC):
                w1c = wpool.tile([128, HT, FC], BF16, tag="w1c")
                nc.sync.dma_start(out=w1c[:], in_=w1_d[b, fc])
                w3c = wpool.tile([128, HT, FC], BF16, tag="w3c")
                nc.sync.dma_start(out=w3c[:], in_=w3_d[b, fc])
                w2c = wpool.tile([128, FT, H], BF16, tag="w2c")
                nc.sync.dma_start(out=w2c[:], in_=w2_d[b, fc])

                gate = pg.tile([128, FC], F32, tag="gate")
                up = pg.tile([128, FC], F32, tag="up")
                for a in range(HT):
                    lhs = xT[:, a, b * TOK:(b + 1) * TOK]
                    nc.tensor.matmul(
                        gate[:], lhs, w1c[:, a, :],
                        start=(a == 0), stop=(a == HT - 1),
                    )
                    nc.tensor.matmul(
                        up[:], lhs, w3c[:, a, :],
                        start=(a == 0), stop=(a == HT - 1),
                    )

                silu = epool.tile([128, FC], F32, tag="silu")
                nc.scalar.activation(silu[:], gate[:], AF.Silu)
                inter = epool.tile([128, FC], BF16, tag="inter")
                nc.vector.tensor_mul(inter[:], silu[:], up[:])

                interT = epool.tile([128, FT, TOK], BF16, tag="interT")
                for ft in range(FT):
                    tps = pt.tile([128, TOK], BF16, tag="tps")
                    nc.tensor.transpose(
                        tps[:], inter[:, ft * 128:(ft + 1) * 128], ident[:]
                    )
                    nc.vector.tensor_copy(interT[:, ft, :], tps[:])

                for ft in range(FT):
                    first = fc == 0 and ft == 0
                    last = fc == N_FC - 1 and ft == FT - 1
                    for n in range(2):
                        nc.tensor.matmul(
                            out_ps[:, n * 512:(n + 1) * 512],
                            interT[:, ft, :],
                            w2c[:, ft, n * 512:(n + 1) * 512],
                            start=first, stop=last,
                        )

            outs = epool.tile([128, H], BF16, tag="outs")
            nc.vector.tensor_copy(outs[:], out_ps[:])
            # Store on the ACT HWDGE ring so a stalled output store can
            # never block (or get resequenced against) the weight stream.
            nc.scalar.dma_start(out=out_d[b * TOK:(b + 1) * TOK, :], in_=outs[:])

    nc.compile()
    return nc


def _get_compiled(nbpc: int):
    if nbpc not in _COMPILED:
        _COMPILED[nbpc] = _build(nbpc)
    return _COMPILED[nbpc]


def _plan_buckets(group_sizes):
    """Split ragged expert groups into <=128-token buckets.

    Returns list of (expert_id, token_start, ntok)."""
    buckets = []
    start = 0
    for e, g in enumerate(np.asarray(group_sizes).astype(np.int64)):
        off = 0
        while off < g:
            n = min(TOK, g - off)
            buckets.append((e, start + off, int(n)))
            off += n
        start += int(g)
    return buckets


def _prepare_in_maps(hidden_states, w1, w3, w2, buckets, nbpc):
    import ml_dtypes

    bf16 = ml_dtypes.bfloat16
    nb = nbpc * N_CORES

    w1b = np.asarray(w1, dtype=bf16)
    w3b = np.asarray(w3, dtype=bf16)
    w2b = np.asarray(w2, dtype=bf16)
    hsb = np.asarray(hidden_states, dtype=bf16)

    # Token buckets: [nb, TOK, H], zero-padded.
    uniform = (
        len(buckets) == nb
        and all(n == TOK for (_, _, n) in buckets)
        and all(s == i * TOK for i, (_, s, _) in enumerate(buckets))
    )
    if uniform:
        xb = hsb.reshape(nb, TOK, H)
        eids = np.array([e for (e, _, _) in buckets])
    else:
        xb = np.zeros((nb, TOK, H), dtype=bf16)
        eids = np.zeros(nb, dtype=np.int64)
        for i, (e, s, n) in enumerate(buckets):
            xb[i, :n] = hsb[s:s + n]
            eids[i] = e

    # Per-bucket weights (gather; identity when one bucket per expert).
    w1g = w1b[eids]  # [nb, H, F]
    w3g = w3b[eids]
    w2g = w2b[eids]  # [nb, F, H]

    # Device layouts (everything contiguous per DMA):
    #  xT [128p(h%128), HT, TPC] per core
    #  wp [b, fc, 128p, w1c(HT,FC) | w3c(HT,FC) | w2c(FT,H)] packed per chunk
    #    w1/w3 partition = h%128, blocks indexed [h//128, f_in_chunk]
    #    w2    partition = f%128, blocks indexed [f//128 within chunk, h]
    w1t = np.ascontiguousarray(
        w1g.reshape(nb, HT, 128, N_FC, FC).transpose(0, 3, 2, 1, 4)
    )
    w3t = np.ascontiguousarray(
        w3g.reshape(nb, HT, 128, N_FC, FC).transpose(0, 3, 2, 1, 4)
    )
    w2t = np.ascontiguousarray(
        w2g.reshape(nb, N_FC, FT, 128, H).transpose(0, 1, 3, 2, 4)
    )

    in_maps = []
    for c in range(N_CORES):
        sl = slice(c * nbpc, (c + 1) * nbpc)
        xc = xb[sl]  # [nbpc, TOK, H]
        # xT: [H, nbpc*TOK] -> [HT, 128, TPC] -> [128, HT, TPC]
        xt = np.ascontiguousarray(
            xc.reshape(nbpc * TOK, H).T.reshape(HT, 128, nbpc * TOK)
            .transpose(1, 0, 2)
        )
        in_maps.append({
            "xt": xt,
            "w1": np.ascontiguousarray(w1t[sl]),
            "w3": np.ascontiguousarray(w3t[sl]),
            "w2": np.ascontiguousarray(w2t[sl]),
        })
    return in_maps


def _run(hidden_states, w1, w3, w2, group_sizes, trace=False, **run_kwargs):
    from concourse.bass_utils import run_bass_kernel_spmd

    buckets = _plan_buckets(group_sizes)
    nbpc = -(-len(buckets) // N_CORES)  # ceil
    nb = nbpc * N_CORES
    while len(buckets) < nb:
        buckets.append((0, 0, 0))  # padding buckets (zero tokens)

    nc = _get_compiled(nbpc)
    in_maps = _prepare_in_maps(hidden_states, w1, w3, w2, buckets, nbpc)
    res = run_bass_kernel_spmd(
        nc, in_maps, core_ids=list(range(N_CORES)), trace=trace, **run_kwargs
    )

    out_buckets = np.concatenate(
        [r["out"].astype(np.float32).reshape(nbpc, TOK, H) for r in res.results],
        axis=0,
    )  # [nb, TOK, H] float32

    T_total = int(np.asarray(group_sizes).sum())
    out = np.zeros((hidden_states.shape[0], H), dtype=np.float32)
    for i, (e, s, n) in enumerate(buckets):
        if n:
            out[s:s + n] = out_buckets[i, :n]
    del T_total
    return out, res


def kernel(hidden_states, w1, w3, w2, group_sizes):
    out, _ = _run(hidden_states, w1, w3, w2, group_sizes)
    return out


# revision 29
# speedup vs baseline: 1.0614x; 1.0516x over previous
"""Trainium2 Bass kernel for ArcticMLP MoE grouped-GEMM (nn_ArcticMLPMoE).

Reference computation (per token group of expert e, tokens sorted by expert):
    gate = x @ w1[e];  up = x @ w3[e];  out = (silu(gate) * up) @ w2[e]

Strategy
--------
Expert-parallel across the 8 NeuronCores: tokens arrive pre-sorted by
expert, so each core owns E/8 experts and their token slices -- zero
collectives.  The problem is weight-DMA bound (each weight byte is used
for only 128 tokens), so on the host we:
  * split tokens into 128-token buckets per expert (general ragged
    group_sizes supported via zero-padding; the standard case of 128
    tokens/expert is a pure reshape),
  * downcast weights/activations to bf16 (halves the HBM traffic;
    matmuls accumulate in fp32 PSUM, norm rel. error ~5e-3 << 2e-2),
  * pre-pack each bucket's weights into one DRAM slab in exact stream
    order, so every chunk DMA reads one fully-contiguous [128, blk]
    block at monotonically increasing addresses.

The device graph is raw Bass (no Tile framework): five engine streams
with hand-placed semaphores, software-pipelined so the weight-DMA queue
never drains.  Per 128-token bucket, streaming w1/w3/w2 in F-chunks:
    gate/up [128t x Wf] = sum_h xT[h,t].T @ w{1,3}[h,f]   (8 k-tiles, PE)
    inter   = silu(gate) * up                             (ACT + DVE)
    interT  [f,t] via PE transpose (identity matmul)
    out    += interT.T @ w2[f,h]    (PSUM accumulation over all chunks)
Chunk widths [512,512,512,256,256]: the small trailing chunks shorten
the serial epilogue tail after the bucket's last weight byte lands.
"""

import sys

import numpy as np

sys.path.insert(0, "/opt/trn_rl_repo")

E = 32
H = 1024
F = 2048
T = 4096
N_CORES = 8
TOK = 128               # tokens per bucket
HT = H // 128           # 8 k-tiles over hidden dim
WIDTHS = [512, 512, 512, 256, 256]   # F-chunk widths per bucket
assert sum(WIDTHS) == F
NCK = len(WIDTHS)       # chunks per bucket
SLAB = 3 * HT * F       # per-partition slab elems per bucket (w1+w3+w2)
SLOT = 3 * HT * 512     # SBUF chunk slot elems/partition (sized for W=512)
W_BUFS = 4              # chunk slots in flight

_COMPILED = {}


def _build(nbpc: int):
    """Build the per-core raw-Bass graph for `nbpc` buckets/core."""
    from contextlib import ExitStack

    import concourse.bass as bass
    import concourse.mybir as mybir

    BF16 = mybir.dt.bfloat16
    F32 = mybir.dt.float32
    AF = mybir.ActivationFunctionType
    TPC = nbpc * TOK
    NK = nbpc * NCK

    # Within-bucket DRAM slab offset of each chunk (same for every bucket).
    cko = []
    o = 0
    for W in WIDTHS:
        cko.append(o)
        o += 2 * HT * W + (W // 128) * H
    assert o == SLAB

    nc = bass.Bass("TRN2", target_bir_lowering=False, debug=False)

    xT_d = nc.dram_tensor("xt", [128, HT, TPC], BF16, kind="ExternalInput")
    w_d = nc.dram_tensor("w", [nbpc, 128, SLAB], BF16, kind="ExternalInput")
    out_d = nc.dram_tensor("out", [TPC, H], BF16, kind="ExternalOutput")

    with ExitStack() as ctx:
        sem = {
            n: ctx.enter_context(nc.semaphore(n))
            for n in ["s_x", "s_wd", "s_id", "s_gu", "s_act", "s_mul",
                      "s_tp", "s_cp", "s_cd", "s_oc", "s_od"]
        }
        s_x, s_wd, s_id = sem["s_x"], sem["s_wd"], sem["s_id"]
        s_gu, s_act, s_mul = sem["s_gu"], sem["s_act"], sem["s_mul"]
        s_tp, s_cp, s_cd = sem["s_tp"], sem["s_cp"], sem["s_cd"]
        s_oc, s_od = sem["s_oc"], sem["s_od"]

        xT = ctx.enter_context(nc.sbuf_tensor("xT", [128, HT * TPC], BF16))
        wbuf = ctx.enter_context(
            nc.sbuf_tensor("wbuf", [128, W_BUFS * SLOT], BF16)
        )
        silu_sb = ctx.enter_context(nc.sbuf_tensor("silu", [128, 2, 512], F32))
        inter_sb = ctx.enter_context(nc.sbuf_tensor("inter", [128, 2, 512], BF16))
        interT_sb = ctx.enter_context(nc.sbuf_tensor("interT", [128, 2, 512], BF16))
        outs_sb = ctx.enter_context(nc.sbuf_tensor("outs", [128, 2, H], BF16))
        ident = ctx.enter_context(nc.sbuf_tensor("ident", [128, 128], BF16))
        zbias = ctx.enter_context(nc.sbuf_tensor("zbias", [128, 1], F32))
        # PSUM: 8 banks exactly.  Double-buffered slots are bank-aligned so
        # a PE write and a DVE/ACT read never touch the same bank.
        gate_ps = ctx.enter_context(nc.psum_tensor("gate", [128, 2, 512], F32))
        up_ps = ctx.enter_context(nc.psum_tensor("up", [128, 2, 512], F32))
        tps_ps = ctx.enter_context(nc.psum_tensor("tps", [128, 2, 1024], BF16))
        out_ps = ctx.enter_context(nc.psum_tensor("ops", [128, H], F32))

        with nc.Block() as block:

            @block.sync
            def _(sync):
                # Weight stream: 3 contiguous DMAs per chunk, throttled only
                # by PE consumption of the slot W_BUFS chunks back.
                for k in range(NK):
                    b = k // NCK
                    W = WIDTHS[k % NCK]
                    sl = (k % W_BUFS) * SLOT
                    if k >= W_BUFS:
                        sync.wait_ge(s_cd, k - W_BUFS + 1)
                    o = cko[k % NCK]
                    n1 = HT * W
                    n2 = (W // 128) * H
                    sync.dma_start(
                        out=wbuf[:, sl:sl + n1], in_=w_d[b][:, o:o + n1]
                    ).then_inc(s_wd, 16)
                    sync.dma_start(
                        out=wbuf[:, sl + n1:sl + 2 * n1],
                        in_=w_d[b][:, o + n1:o + 2 * n1],
                    ).then_inc(s_wd, 16)
                    sync.dma_start(
                        out=wbuf[:, sl + 2 * n1:sl + 2 * n1 + n2],
                        in_=w_d[b][:, o + 2 * n1:o + 2 * n1 + n2],
                    ).then_inc(s_wd, 16)

            @block.scalar
            def _(scalar):
                scalar.dma_start(out=xT[:, :], in_=xT_d[:]).then_inc(s_x, 16)
                scalar.wait_ge(s_id, 1)  # zbias memset done (gpsimd)
                for k in range(NK):
                    W = WIDTHS[k % NCK]
                    scalar.wait_ge(s_gu, k + 1)
                    if k >= 2:
                        scalar.wait_ge(s_mul, k - 1)  # silu slot WAR
                    scalar.activation(
                        silu_sb[:, k % 2, :W], gate_ps[:, k % 2, :W],
                        AF.Silu, bias=zbias[:, :],
                    ).then_inc(s_act)

            @block.vector
            def _(vector):
                for k in range(NK):
                    b, c = divmod(k, NCK)
                    W = WIDTHS[c]
                    vector.wait_ge(s_act, k + 1)
                    if k >= 2:
                        vector.wait_ge(s_tp, k - 1)  # inter slot WAR
                    vector.tensor_mul(
                        inter_sb[:, k % 2, :W], silu_sb[:, k % 2, :W],
                        up_ps[:, k % 2, :W],
                    ).then_inc(s_mul)
                    vector.wait_ge(s_tp, k + 1)
                    if k >= 2:
                        vector.wait_ge(s_cd, k - 1)  # interT slot WAR
                    vector.tensor_copy(
                        interT_sb[:, k % 2, :W], tps_ps[:, k % 2, :W]
                    ).then_inc(s_cp)
                    if c == NCK - 1:
                        vector.wait_ge(s_cd, NCK * (b + 1))
                        if b >= 2:
                            vector.wait_ge(s_od, 16 * (b - 1))  # outs slot WAR
                        vector.tensor_copy(
                            outs_sb[:, b % 2, :], out_ps[:, :]
                        ).then_inc(s_oc)

            @block.tensor
            def _(t):
                # Software pipeline: gate/up matmuls of chunk k, then the
                # epilogue (transposes + down-proj) of chunk k-1, so the
                # ACT/DVE latency of chunk k-1 hides under chunk k's work
                # and the weight-DMA wait.
                t.wait_ge(s_x, 16)
                for k in range(NK + 1):
                    if k < NK:
                        b, c = divmod(k, NCK)
                        W = WIDTHS[c]
                        sl = (k % W_BUFS) * SLOT
                        n1 = HT * W
                        t.wait_ge(s_wd, 48 * k + 32)  # w1c+w3c of chunk k in
                        if k >= 2:
                            t.wait_ge(s_act, k - 1)   # gate bank WAR
                            t.wait_ge(s_mul, k - 1)   # up bank WAR
                        for a in range(HT):
                            lhs = xT[:, a * TPC + b * TOK:
                                     a * TPC + (b + 1) * TOK]
                            t.matmul(
                                gate_ps[:, k % 2, :W], lhs,
                                wbuf[:, sl + a * W:sl + (a + 1) * W],
                                start=(a == 0), stop=(a == HT - 1),
                            )
                            mm = t.matmul(
                                up_ps[:, k % 2, :W], lhs,
                                wbuf[:, sl + n1 + a * W:sl + n1 + (a + 1) * W],
                                start=(a == 0), stop=(a == HT - 1),
                            )
                        mm.then_inc(s_gu)
                    if k >= 1:
                        j = k - 1
                        jb, jc = divmod(j, NCK)
                        Wj = WIDTHS[jc]
                        WTj = Wj // 128
                        slj = (j % W_BUFS) * SLOT
                        n1j = HT * Wj
                        if j == 0:
                            t.wait_ge(s_id, 1)        # identity ready
                        t.wait_ge(s_mul, j + 1)       # inter ready
                        if j >= 2:
                            t.wait_ge(s_cp, j - 1)    # tps bank WAR
                        for ft in range(WTj):
                            tr = t.transpose(
                                tps_ps[:, j % 2, ft * 128:(ft + 1) * 128],
                                inter_sb[:, j % 2, ft * 128:(ft + 1) * 128],
                                ident[:, :],
                            )
                        tr.then_inc(s_tp)
                        t.wait_ge(s_cp, j + 1)        # interT copied
                        if jc == 0 and jb >= 1:
                            t.wait_ge(s_oc, jb)       # out_ps WAR vs outs copy
                        for ft in range(WTj):
                            for n in range(2):
                                w2o = slj + 2 * n1j + ft * H + n * 512
                                mm = t.matmul(
                                    out_ps[:, n * 512:(n + 1) * 512],
                                    interT_sb[:, j % 2,
                                              ft * 128:(ft + 1) * 128],
                                    wbuf[:, w2o:w2o + 512],
                                    start=(jc == 0 and ft == 0),
                                    stop=(jc == NCK - 1 and ft == WTj - 1),
                                )
                        mm.then_inc(s_cd)

            @block.gpsimd
            def _(gpsimd):
                import concourse.mybir as _mb
                gpsimd.memset(zbias[:, :], 0.0)
                gpsimd.memset(ident[:, :], 0.0)
                gpsimd.affine_select(
                    out=ident[:, :], in_=ident[:, :],
                    compare_op=_mb.AluOpType.not_equal,
                    fill=1.0, base=0, pattern=[[-1, 128]],
                    channel_multiplier=1,
                ).then_inc(s_id)
                # Output stores ride SWDGE: fully off the weight ring.
                for b in range(nbpc):
                    gpsimd.wait_ge(s_oc, b + 1)
                    gpsimd.dma_start(
                        out=out_d[b * TOK:(b + 1) * TOK, :],
                        in_=outs_sb[:, b % 2, :],
                    ).then_inc(s_od, 16)
                gpsimd.wait_ge(s_od, 16 * nbpc)

    return nc


def _get_compiled(nbpc: int):
    if nbpc not in _COMPILED:
        _COMPILED[nbpc] = _build(nbpc)
    return _COMPILED[nbpc]


def _plan_buckets(group_sizes):
    """Split ragged expert groups into <=128-token buckets.

    Returns list of (expert_id, token_start, ntok)."""
    buckets = []
    start = 0
    for e, g in enumerate(np.asarray(group_sizes).astype(np.int64)):
        off = 0
        while off < g:
            n = min(TOK, g - off)
            buckets.append((e, start + off, int(n)))
            off += n
        start += int(g)
    return buckets


def _prepare_in_maps(hidden_states, w1, w3, w2, buckets, nbpc):
    import ml_dtypes

    bf16 = ml_dtypes.bfloat16
    nb = nbpc * N_CORES

    w1b = np.asarray(w1, dtype=bf16)
    w3b = np.asarray(w3, dtype=bf16)
    w2b = np.asarray(w2, dtype=bf16)
    hsb = np.asarray(hidden_states, dtype=bf16)

    # Token buckets: [nb, TOK, H], zero-padded.
    uniform = (
        len(buckets) == nb
        and all(n == TOK for (_, _, n) in buckets)
        and all(s == i * TOK for i, (_, s, _) in enumerate(buckets))
    )
    if uniform:
        xb = hsb.reshape(nb, TOK, H)
        eids = np.array([e for (e, _, _) in buckets])
    else:
        xb = np.zeros((nb, TOK, H), dtype=bf16)
        eids = np.zeros(nb, dtype=np.int64)
        for i, (e, s, n) in enumerate(buckets):
            xb[i, :n] = hsb[s:s + n]
            eids[i] = e

    # Per-bucket weights (gather; identity when one bucket per expert).
    w1g = w1b[eids]  # [nb, H, F]
    w3g = w3b[eids]
    w2g = w2b[eids]  # [nb, F, H]

    # Device layouts:
    #  xT [128p(h%128), HT, TPC] per core
    #  w  [nb, 128p, concat over chunks of [w1c(HT,W) | w3c(HT,W) | w2c(W/128,H)]]
    #     (w1/w3 blocks: partition = h%128; w2 blocks: partition = f%128)
    blks = []
    f0 = 0
    for W in WIDTHS:
        blks.append(
            w1g[:, :, f0:f0 + W].reshape(nb, HT, 128, W)
            .transpose(0, 2, 1, 3).reshape(nb, 128, HT * W)
        )
        blks.append(
            w3g[:, :, f0:f0 + W].reshape(nb, HT, 128, W)
            .transpose(0, 2, 1, 3).reshape(nb, 128, HT * W)
        )
        blks.append(
            w2g[:, f0:f0 + W, :].reshape(nb, W // 128, 128, H)
            .transpose(0, 2, 1, 3).reshape(nb, 128, (W // 128) * H)
        )
        f0 += W
    wt = np.concatenate(blks, axis=2)

    in_maps = []
    for c in range(N_CORES):
        sl = slice(c * nbpc, (c + 1) * nbpc)
        xc = xb[sl]  # [nbpc, TOK, H]
        # xT: [H, nbpc*TOK] -> [HT, 128, TPC] -> [128, HT, TPC]
        xt = np.ascontiguousarray(
            xc.reshape(nbpc * TOK, H).T.reshape(HT, 128, nbpc * TOK)
            .transpose(1, 0, 2)
        )
        in_maps.append({
            "xt": xt,
            "w": np.ascontiguousarray(wt[sl]),
        })
    return in_maps


def _run(hidden_states, w1, w3, w2, group_sizes, trace=False, **run_kwargs):
    from concourse.bass_utils import run_bass_kernel_spmd

    buckets = _plan_buckets(group_sizes)
    nbpc = -(-len(buckets) // N_CORES)  # ceil
    nb = nbpc * N_CORES
    while len(buckets) < nb:
        buckets.append((0, 0, 0))  # padding buckets (zero tokens)

    nc = _get_compiled(nbpc)
    in_maps = _prepare_in_maps(hidden_states, w1, w3, w2, buckets, nbpc)
    res = run_bass_kernel_spmd(
        nc, in_maps, core_ids=list(range(N_CORES)), trace=trace, **run_kwargs
    )

    out_buckets = np.concatenate(
        [r["out"].astype(np.float32).reshape(nbpc, TOK, H) for r in res.results],
        axis=0,
    )  # [nb, TOK, H] float32

    out = np.zeros((hidden_states.shape[0], H), dtype=np.float32)
    for i, (e, s, n) in enumerate(buckets):
        if n:
            out[s:s + n] = out_buckets[i, :n]
    return out, res


def kernel(hidden_states, w1, w3, w2, group_sizes):
    out, _ = _run(hidden_states, w1, w3, w2, group_sizes)
    return out


# revision 31
# speedup vs baseline: 1.0773x; 1.0149x over previous
"""Trainium2 Bass kernel for ArcticMLP MoE grouped-GEMM (nn_ArcticMLPMoE).

Reference computation (per token group of expert e, tokens sorted by expert):
    gate = x @ w1[e];  up = x @ w3[e];  out = (silu(gate) * up) @ w2[e]

Strategy
--------
Expert-parallel across the 8 NeuronCores: tokens arrive pre-sorted by
expert, so each core owns E/8 experts and their token slices -- zero
collectives.  The problem is weight-DMA bound (each weight byte is used
for only 128 tokens), so on the host we:
  * split tokens into 128-token buckets per expert (general ragged
    group_sizes supported via zero-padding; the standard case of 128
    tokens/expert is a pure reshape),
  * downcast weights/activations to bf16 (halves the HBM traffic;
    matmuls accumulate in fp32 PSUM, norm rel. error ~5e-3 << 2e-2),
  * pre-pack each bucket's weights into one DRAM slab in exact stream
    order, so every chunk DMA reads one fully-contiguous [128, blk]
    block at monotonically increasing addresses.

The device graph is raw Bass (no Tile framework): five engine streams
with hand-placed semaphores, software-pipelined so the weight-DMA queue
never drains.  Per 128-token bucket, streaming w1/w3/w2 in F-chunks:
    gate/up [128t x Wf] = sum_h xT[h,t].T @ w{1,3}[h,f]   (8 k-tiles, PE)
    inter   = silu(gate) * up                             (ACT + DVE)
    interT  [f,t] via PE transpose (identity matmul)
    out    += interT.T @ w2[f,h]    (PSUM accumulation over all chunks)
Chunk widths [512,512,512,256,256]: the small trailing chunks shorten
the serial epilogue tail after the bucket's last weight byte lands.
"""

import sys

import numpy as np

sys.path.insert(0, "/opt/trn_rl_repo")

E = 32
H = 1024
F = 2048
T = 4096
N_CORES = 8
TOK = 128               # tokens per bucket
HT = H // 128           # 8 k-tiles over hidden dim
WIDTHS = [512, 512, 512, 256, 256]   # F-chunk widths per bucket
assert sum(WIDTHS) == F
NCK = len(WIDTHS)       # chunks per bucket
SLAB = 3 * HT * F       # per-partition slab elems per bucket (w1+w3+w2)
SLOT = 3 * HT * 512     # SBUF chunk slot elems/partition (sized for W=512)
W_BUFS = 4              # chunk slots in flight

_COMPILED = {}


def _build(nbpc: int):
    """Build the per-core raw-Bass graph for `nbpc` buckets/core."""
    from contextlib import ExitStack

    import concourse.bass as bass
    import concourse.mybir as mybir

    BF16 = mybir.dt.bfloat16
    F32 = mybir.dt.float32
    AF = mybir.ActivationFunctionType
    TPC = nbpc * TOK
    NK = nbpc * NCK

    # Within-bucket DRAM slab offset of each chunk (same for every bucket).
    cko = []
    o = 0
    for W in WIDTHS:
        cko.append(o)
        o += 2 * HT * W + (W // 128) * H
    assert o == SLAB

    nc = bass.Bass("TRN2", target_bir_lowering=False, debug=False)

    xT_d = nc.dram_tensor("xt", [128, HT, TPC], BF16, kind="ExternalInput")
    w_d = nc.dram_tensor("w", [nbpc, 128, SLAB], BF16, kind="ExternalInput")
    out_d = nc.dram_tensor("out", [TPC, H], BF16, kind="ExternalOutput")

    with ExitStack() as ctx:
        sem = {
            n: ctx.enter_context(nc.semaphore(n))
            for n in ["s_x", "s_wd", "s_id", "s_gu", "s_act", "s_mul",
                      "s_tp", "s_cp", "s_cd", "s_oc", "s_od"]
        }
        s_x, s_wd, s_id = sem["s_x"], sem["s_wd"], sem["s_id"]
        s_gu, s_act, s_mul = sem["s_gu"], sem["s_act"], sem["s_mul"]
        s_tp, s_cp, s_cd = sem["s_tp"], sem["s_cp"], sem["s_cd"]
        s_oc, s_od = sem["s_oc"], sem["s_od"]

        xT = ctx.enter_context(nc.sbuf_tensor("xT", [128, HT * TPC], BF16))
        wbuf = ctx.enter_context(
            nc.sbuf_tensor("wbuf", [128, W_BUFS * SLOT], BF16)
        )
        silu_sb = ctx.enter_context(nc.sbuf_tensor("silu", [128, 2, 512], F32))
        inter_sb = ctx.enter_context(nc.sbuf_tensor("inter", [128, 2, 512], BF16))
        interT_sb = ctx.enter_context(nc.sbuf_tensor("interT", [128, 2, 512], BF16))
        outs_sb = ctx.enter_context(nc.sbuf_tensor("outs", [128, 2, H], BF16))
        ident = ctx.enter_context(nc.sbuf_tensor("ident", [128, 128], BF16))
        zbias = ctx.enter_context(nc.sbuf_tensor("zbias", [128, 1], F32))
        # PSUM: 8 banks exactly.  Double-buffered slots are bank-aligned so
        # a PE write and a DVE/ACT read never touch the same bank.
        gate_ps = ctx.enter_context(nc.psum_tensor("gate", [128, 2, 512], F32))
        up_ps = ctx.enter_context(nc.psum_tensor("up", [128, 2, 512], F32))
        tps_ps = ctx.enter_context(nc.psum_tensor("tps", [128, 2, 1024], BF16))
        out_ps = ctx.enter_context(nc.psum_tensor("ops", [128, H], F32))

        with nc.Block() as block:

            @block.sync
            def _(sync):
                # Weight stream: 3 contiguous DMAs per chunk, throttled only
                # by PE consumption of the slot W_BUFS chunks back.
                for k in range(NK):
                    b = k // NCK
                    W = WIDTHS[k % NCK]
                    sl = (k % W_BUFS) * SLOT
                    if k >= W_BUFS:
                        sync.wait_ge(s_cd, k - W_BUFS + 1)
                    o = cko[k % NCK]
                    n1 = HT * W
                    n2 = (W // 128) * H
                    sync.dma_start(
                        out=wbuf[:, sl:sl + n1], in_=w_d[b][:, o:o + n1]
                    ).then_inc(s_wd, 16)
                    sync.dma_start(
                        out=wbuf[:, sl + n1:sl + 2 * n1],
                        in_=w_d[b][:, o + n1:o + 2 * n1],
                    ).then_inc(s_wd, 16)
                    sync.dma_start(
                        out=wbuf[:, sl + 2 * n1:sl + 2 * n1 + n2],
                        in_=w_d[b][:, o + 2 * n1:o + 2 * n1 + n2],
                    ).then_inc(s_wd, 16)

            @block.scalar
            def _(scalar):
                scalar.dma_start(out=xT[:, :], in_=xT_d[:]).then_inc(s_x, 16)
                scalar.wait_ge(s_id, 1)  # zbias memset done (gpsimd)
                for k in range(NK):
                    b, c = divmod(k, NCK)
                    W = WIDTHS[c]
                    scalar.wait_ge(s_gu, k + 1)
                    if k >= 2:
                        scalar.wait_ge(s_mul, k - 1)  # silu slot WAR
                    scalar.activation(
                        silu_sb[:, k % 2, :W], gate_ps[:, k % 2, :W],
                        AF.Silu, bias=zbias[:, :],
                    ).then_inc(s_act)
                    if c == NCK - 1:
                        scalar.wait_ge(s_oc, b + 1)
                        scalar.dma_start(
                            out=out_d[b * TOK:(b + 1) * TOK, :],
                            in_=outs_sb[:, b % 2, :],
                        ).then_inc(s_od, 16)
                scalar.wait_ge(s_od, 16 * nbpc)

            @block.vector
            def _(vector):
                for k in range(NK):
                    b, c = divmod(k, NCK)
                    W = WIDTHS[c]
                    vector.wait_ge(s_act, k + 1)
                    if k >= 2:
                        vector.wait_ge(s_tp, k - 1)  # inter slot WAR
                    vector.tensor_mul(
                        inter_sb[:, k % 2, :W], silu_sb[:, k % 2, :W],
                        up_ps[:, k % 2, :W],
                    ).then_inc(s_mul)
                    vector.wait_ge(s_tp, k + 1)
                    if k >= 2:
                        vector.wait_ge(s_cd, k - 1)  # interT slot WAR
                    vector.tensor_copy(
                        interT_sb[:, k % 2, :W], tps_ps[:, k % 2, :W]
                    ).then_inc(s_cp)
                    if c == NCK - 1:
                        vector.wait_ge(s_cd, NCK * (b + 1))
                        if b >= 2:
                            vector.wait_ge(s_od, 16 * (b - 1))  # outs slot WAR
                        vector.tensor_copy(
                            outs_sb[:, b % 2, :], out_ps[:, :]
                        ).then_inc(s_oc)

            @block.tensor
            def _(t):
                # Software pipeline: gate/up matmuls of chunk k, then the
                # epilogue (transposes + down-proj) of chunk k-1, so the
                # ACT/DVE latency of chunk k-1 hides under chunk k's work
                # and the weight-DMA wait.
                t.wait_ge(s_x, 16)
                for k in range(NK + 1):
                    if k < NK:
                        b, c = divmod(k, NCK)
                        W = WIDTHS[c]
                        sl = (k % W_BUFS) * SLOT
                        n1 = HT * W
                        t.wait_ge(s_wd, 48 * k + 32)  # w1c+w3c of chunk k in
                        if k >= 2:
                            t.wait_ge(s_act, k - 1)   # gate bank WAR
                            t.wait_ge(s_mul, k - 1)   # up bank WAR
                        for a in range(HT):
                            lhs = xT[:, a * TPC + b * TOK:
                                     a * TPC + (b + 1) * TOK]
                            t.matmul(
                                gate_ps[:, k % 2, :W], lhs,
                                wbuf[:, sl + a * W:sl + (a + 1) * W],
                                start=(a == 0), stop=(a == HT - 1),
                            )
                            mm = t.matmul(
                                up_ps[:, k % 2, :W], lhs,
                                wbuf[:, sl + n1 + a * W:sl + n1 + (a + 1) * W],
                                start=(a == 0), stop=(a == HT - 1),
                            )
                        mm.then_inc(s_gu)
                    if k >= 1:
                        j = k - 1
                        jb, jc = divmod(j, NCK)
                        Wj = WIDTHS[jc]
                        WTj = Wj // 128
                        slj = (j % W_BUFS) * SLOT
                        n1j = HT * Wj
                        if j == 0:
                            t.wait_ge(s_id, 1)        # identity ready
                        t.wait_ge(s_mul, j + 1)       # inter ready
                        if j >= 2:
                            t.wait_ge(s_cp, j - 1)    # tps bank WAR
                        for ft in range(WTj):
                            tr = t.transpose(
                                tps_ps[:, j % 2, ft * 128:(ft + 1) * 128],
                                inter_sb[:, j % 2, ft * 128:(ft + 1) * 128],
                                ident[:, :],
                            )
                        tr.then_inc(s_tp)
                        t.wait_ge(s_cp, j + 1)        # interT copied
                        t.wait_ge(s_wd, 48 * (j + 1))  # w2c of chunk j landed
                        if jc == 0 and jb >= 1:
                            t.wait_ge(s_oc, jb)       # out_ps WAR vs outs copy
                        for ft in range(WTj):
                            for n in range(2):
                                w2o = slj + 2 * n1j + ft * H + n * 512
                                mm = t.matmul(
                                    out_ps[:, n * 512:(n + 1) * 512],
                                    interT_sb[:, j % 2,
                                              ft * 128:(ft + 1) * 128],
                                    wbuf[:, w2o:w2o + 512],
                                    start=(jc == 0 and ft == 0),
                                    stop=(jc == NCK - 1 and ft == WTj - 1),
                                )
                        mm.then_inc(s_cd)

            @block.gpsimd
            def _(gpsimd):
                import concourse.mybir as _mb
                gpsimd.memset(zbias[:, :], 0.0)
                gpsimd.memset(ident[:, :], 0.0)
                gpsimd.affine_select(
                    out=ident[:, :], in_=ident[:, :],
                    compare_op=_mb.AluOpType.not_equal,
                    fill=1.0, base=0, pattern=[[-1, 128]],
                    channel_multiplier=1,
                ).then_inc(s_id)
                gpsimd.wait_ge(s_od, 16 * nbpc)

    return nc


def _get_compiled(nbpc: int):
    if nbpc not in _COMPILED:
        _COMPILED[nbpc] = _build(nbpc)
    return _COMPILED[nbpc]


def _plan_buckets(group_sizes):
    """Split ragged expert groups into <=128-token buckets.

    Returns list of (expert_id, token_start, ntok)."""
    buckets = []
    start = 0
    for e, g in enumerate(np.asarray(group_sizes).astype(np.int64)):
        off = 0
        while off < g:
            n = min(TOK, g - off)
            buckets.append((e, start + off, int(n)))
            off += n
        start += int(g)
    return buckets


def _prepare_in_maps(hidden_states, w1, w3, w2, buckets, nbpc):
    import ml_dtypes

    bf16 = ml_dtypes.bfloat16
    nb = nbpc * N_CORES

    w1b = np.asarray(w1, dtype=bf16)
    w3b = np.asarray(w3, dtype=bf16)
    w2b = np.asarray(w2, dtype=bf16)
    hsb = np.asarray(hidden_states, dtype=bf16)

    # Token buckets: [nb, TOK, H], zero-padded.
    uniform = (
        len(buckets) == nb
        and all(n == TOK for (_, _, n) in buckets)
        and all(s == i * TOK for i, (_, s, _) in enumerate(buckets))
    )
    if uniform:
        xb = hsb.reshape(nb, TOK, H)
        eids = np.array([e for (e, _, _) in buckets])
    else:
        xb = np.zeros((nb, TOK, H), dtype=bf16)
        eids = np.zeros(nb, dtype=np.int64)
        for i, (e, s, n) in enumerate(buckets):
            xb[i, :n] = hsb[s:s + n]
            eids[i] = e

    # Per-bucket weights (gather; identity when one bucket per expert).
    w1g = w1b[eids]  # [nb, H, F]
    w3g = w3b[eids]
    w2g = w2b[eids]  # [nb, F, H]

    # Device layouts:
    #  xT [128p(h%128), HT, TPC] per core
    #  w  [nb, 128p, concat over chunks of [w1c(HT,W) | w3c(HT,W) | w2c(W/128,H)]]
    #     (w1/w3 blocks: partition = h%128; w2 blocks: partition = f%128)
    blks = []
    f0 = 0
    for W in WIDTHS:
        blks.append(
            w1g[:, :, f0:f0 + W].reshape(nb, HT, 128, W)
            .transpose(0, 2, 1, 3).reshape(nb, 128, HT * W)
        )
        blks.append(
            w3g[:, :, f0:f0 + W].reshape(nb, HT, 128, W)
            .transpose(0, 2, 1, 3).reshape(nb, 128, HT * W)
        )
        blks.append(
            w2g[:, f0:f0 + W, :].reshape(nb, W // 128, 128, H)
            .transpose(0, 2, 1, 3).reshape(nb, 128, (W // 128) * H)
        )
        f0 += W
    wt = np.concatenate(blks, axis=2)

    in_maps = []
    for c in range(N_CORES):
        sl = slice(c * nbpc, (c + 1) * nbpc)
        xc = xb[sl]  # [nbpc, TOK, H]
        # xT: [H, nbpc*TOK] -> [HT, 128, TPC] -> [128, HT, TPC]
        xt = np.ascontiguousarray(
            xc.reshape(nbpc * TOK, H).T.reshape(HT, 128, nbpc * TOK)
            .transpose(1, 0, 2)
        )
        in_maps.append({
            "xt": xt,
            "w": np.ascontiguousarray(wt[sl]),
        })
    return in_maps


def _run(hidden_states, w1, w3, w2, group_sizes, trace=False, **run_kwargs):
    from concourse.bass_utils import run_bass_kernel_spmd

    buckets = _plan_buckets(group_sizes)
    nbpc = -(-len(buckets) // N_CORES)  # ceil
    nb = nbpc * N_CORES
    while len(buckets) < nb:
        buckets.append((0, 0, 0))  # padding buckets (zero tokens)

    nc = _get_compiled(nbpc)
    in_maps = _prepare_in_maps(hidden_states, w1, w3, w2, buckets, nbpc)
    res = run_bass_kernel_spmd(
        nc, in_maps, core_ids=list(range(N_CORES)), trace=trace, **run_kwargs
    )

    out_buckets = np.concatenate(
        [r["out"].astype(np.float32).reshape(nbpc, TOK, H) for r in res.results],
        axis=0,
    )  # [nb, TOK, H] float32

    out = np.zeros((hidden_states.shape[0], H), dtype=np.float32)
    for i, (e, s, n) in enumerate(buckets):
        if n:
            out[s:s + n] = out_buckets[i, :n]
    return out, res


def kernel(hidden_states, w1, w3, w2, group_sizes):
    out, _ = _run(hidden_states, w1, w3, w2, group_sizes)
    return out
